# revision 51
# baseline (speedup 1.0000x reference)
"""Trainium2 Bass kernel for nn_MemTransformerLM (Transformer-XL layer).

Sharding (8 cores): batch (4) x head-half (2). Core c handles batch b = c//2
and heads [hh*8, hh*8+8), hh = c%2, for all 1024 queries. After o_proj a
2-rank bf16 ReduceScatter over core pairs (2b, 2b+1) splits tokens for the
FFN: even core keeps tokens [0,512), odd [512,1024).

Key structure:
- All big f32 DRAM inputs are loaded with SWDGE cast-DMA (f32 -> bf16).
- r^T / x^T built via SBUF->SBUF DMA transpose in key-quarters so projection
  matmuls start while later quarters still stream.
- Scores per (head h, q-tile s): BD window matmuls -> PSUM -> drain to bdw
  (window buffer, masked tail memset to -240); AC matmuls -> PSUM -> drain to
  sb; a "diagonal" SBUF->SBUF accum-DMA adds the rel-shifted BD window onto
  sb; one ACT pass computes exp(scale * sb) in place; DMA-transpose into a
  per-s slab; PV accumulates slab tiles over the unmasked j range only.
- o_proj and FFN run in natural token-major orientation (attnT / hT used as
  lhsT), so no fp32 PE transposes and no strided weight loads.
"""

import contextlib
import math

import numpy as np

import concourse.bass as bass
import concourse.bacc as bacc
import concourse.mybir as mybir
import concourse.tile as tile
from concourse.masks import make_identity

F32 = mybir.dt.float32
BF16 = mybir.dt.bfloat16
AF = mybir.ActivationFunctionType
ALU = mybir.AluOpType


class Cfg:
    D = 1024      # model dim
    NHC = 8       # heads per core
    DH = 64       # head dim
    KL = 2048     # key length
    Q = 1024      # query length
    DI = 4096     # ffn inner
    LN_EPS = 1e-5
    N_CORES = 8

    HD = 512          # head dims per core (NHC * DH)
    M = 1024          # mem length
    TOKF = 512        # ffn tokens per core
    WB = 2176         # bdw window buffer width (KL + 128)
    SCALE = 0.125     # 1/sqrt(DH)
    MASKV = -240.0    # pre-scale mask value: exp(SCALE * -240) ~ 1e-13
    DPT = 8           # D / 128
    HPT = 4           # HD / 128
    NTT = 16          # KL / 128
    VW = 520          # NHC * 65

    def jmax(self, s):
        return 1152 + 128 * s

    def wstart(self, s):
        return 896 - 128 * s


def ts(i, n):
    return slice(i * n, (i + 1) * n)


def chunks(total, sz=512):
    return [(lo, min(total, lo + sz)) for lo in range(0, total, sz)]


def build_kernel(c: Cfg = None, collective=True):
    c = c or Cfg()
    nc = bacc.Bacc("TRN2", target_bir_lowering=False)

    io = {}
    def din(name, shape, dt=F32):
        io[name] = nc.dram_tensor(name, shape, dt, kind="ExternalInput")
    din("xw", [c.KL, c.D])
    din("r_in", [c.KL, c.D])
    din("qkvw", [c.D, 3 * c.HD])
    din("rnetw", [c.D, c.HD])
    din("oww", [c.HD, c.D])
    din("rwb", [1, c.HD])
    din("rrb", [1, c.HD])
    din("ln1g", [1, c.D]); din("ln1b", [1, c.D])
    din("ln2g", [1, c.D]); din("ln2b", [1, c.D])
    din("ffw1", [c.D, c.DI]); din("ffb1", [1, c.DI])
    din("ffw2", [c.DI, c.D]); din("ffb2", [1, c.D])
    din("wres", [c.TOKF, c.D])
    io["out"] = nc.dram_tensor("out", [c.TOKF, c.D], F32, kind="ExternalOutput")
    io["rs_bin"] = nc.dram_tensor("rs_bin", [c.Q, c.D], BF16)
    io["rs_bout"] = nc.dram_tensor("rs_bout", [c.TOKF, c.D], BF16)

    with tile.TileContext(nc) as tc:
        _body(tc, nc, c, io, collective=collective)
    nc.finalize()
    return nc


def _body(tc, nc, c, io, collective=True):
    ctx = contextlib.ExitStack()
    rg = [[i, i + 1] for i in range(0, c.N_CORES, 2)]
    with ctx:
        small = ctx.enter_context(tc.tile_pool(name="small", bufs=4))
        psA = ctx.enter_context(tc.tile_pool(name="psA", bufs=3, space="PSUM"))
        psB = ctx.enter_context(tc.tile_pool(name="psB", bufs=3, space="PSUM"))
        psPV = ctx.enter_context(tc.tile_pool(name="psPV", bufs=2, space="PSUM"))

        keep = ctx.enter_context(tc.tile_pool(name="keep", bufs=1))
        ident = keep.tile([128, 128], BF16, tag="identb")
        make_identity(nc, ident)
        identf = keep.tile([128, 128], F32, tag="identf")
        make_identity(nc, identf)

        # biases: rwb/rrb as [128, HPT] (partition = dh within head pair col)
        rwb_s = keep.tile([128, c.HPT], F32, tag="rwb")
        rrb_s = keep.tile([128, c.HPT], F32, tag="rrb")

        # pools created in stack order (release: attk -> atp; rest at exit)
        wo = ctx.enter_context(tc.tile_pool(name="wo", bufs=1))
        wff1 = ctx.enter_context(tc.tile_pool(name="wff1", bufs=1))
        wff2 = ctx.enter_context(tc.tile_pool(name="wff2", bufs=1))
        atp = tc.alloc_tile_pool(name="atp", bufs=1)
        attnT = atp.tile([128, c.HPT * c.Q], BF16, tag="attnT")
        # long-lived attention operands
        attk = tc.alloc_tile_pool(name="attk", bufs=1)
        rTp = attk.tile([128, c.HPT * c.KL], BF16, tag="rTp")
        kT = attk.tile([128, c.HPT * c.KL], BF16, tag="kT")
        vb = attk.tile([128, c.NTT * c.VW], BF16, tag="vb")
        rwq = attk.tile([128, c.HPT * c.Q], BF16, tag="rwq")
        rrq = attk.tile([128, c.HPT * c.Q], BF16, tag="rrq")

        # ============ phase AB: loads + projections, pipelined ============
        with tc.tile_pool(name="phAB", bufs=1) as phAB, \
             tc.tile_pool(name="stAB", bufs=3) as stAB:
            rTq = [phAB.tile([128, c.DPT * 512], BF16, tag="rTq%d" % q,
                             name="rTq%d" % q) for q in range(4)]
            xTq = [phAB.tile([128, c.DPT * 512], BF16, tag="xTq%d" % q,
                             name="xTq%d" % q) for q in range(4)]
            wr = phAB.tile([128, c.DPT * c.HD], BF16, tag="wr")
            wqkv = phAB.tile([128, c.DPT * 3 * c.HD], BF16, tag="wqkv")

            nc.gpsimd.dma_start(out=wr[:], in_=bass.AP(
                tensor=io["rnetw"].ap().tensor, offset=0,
                ap=[[c.HD, 128], [128 * c.HD, c.DPT], [1, c.HD]]))
            nc.sync.dma_start(out=rwb_s[:], in_=bass.AP(
                tensor=io["rwb"].ap().tensor, offset=0, ap=[[1, 128], [128, c.HPT]]))
            nc.sync.dma_start(out=rrb_s[:], in_=bass.AP(
                tensor=io["rrb"].ap().tensor, offset=0, ap=[[1, 128], [128, c.HPT]]))
            nc.gpsimd.dma_start(out=wqkv[:], in_=bass.AP(
                tensor=io["qkvw"].ap().tensor, offset=0,
                ap=[[3 * c.HD, 128], [128 * 3 * c.HD, c.DPT], [1, 3 * c.HD]]))

            def load_transposed(src, dstq):
                # 8 cast-DMAs of 2 key-tiles each; transpose per key-tile
                for g in range(8):
                    nat = stAB.tile([128, 2 * c.D], BF16, tag="nat")
                    nc.gpsimd.dma_start(out=nat[:], in_=bass.AP(
                        tensor=src.ap().tensor, offset=g * 2 * 128 * c.D,
                        ap=[[c.D, 128], [128 * c.D, 2], [1, c.D]]))
                    for h2 in range(2):
                        tt = g * 2 + h2
                        dst = dstq[tt // 4]
                        dstap = bass.AP(
                            tensor=dst.tensor,
                            offset=dst.offset + (tt % 4) * 128,
                            ap=[[c.DPT * 512, 128], [512, c.DPT], [1, 128]])
                        nc.sync.dma_start(
                            out=dstap, in_=nat[:, ts(h2, c.D)], transpose=True)

            load_transposed(io["r_in"], rTq)
            load_transposed(io["xw"], xTq)

            # rTp: r_head_k^T per head-pair m
            for m in range(c.HPT):
                for q4 in range(4):
                    ps = psA.tile([128, 512], F32, tag="a", name="psa")
                    for k in range(c.DPT):
                        nc.tensor.matmul(
                            ps[:], wr[:, k * c.HD + m * 128: k * c.HD + (m + 1) * 128],
                            rTq[q4][:, ts(k, 512)],
                            start=(k == 0), stop=(k == c.DPT - 1))
                    nc.vector.tensor_copy(
                        out=rTp[:, m * c.KL + q4 * 512: m * c.KL + (q4 + 1) * 512],
                        in_=ps[:])
            # K^T
            for m in range(c.HPT):
                for q4 in range(4):
                    ps = psB.tile([128, 512], F32, tag="b", name="psb")
                    for k in range(c.DPT):
                        nc.tensor.matmul(
                            ps[:], wqkv[:, k * 1536 + c.HD + m * 128:
                                        k * 1536 + c.HD + (m + 1) * 128],
                            xTq[q4][:, ts(k, 512)],
                            start=(k == 0), stop=(k == c.DPT - 1))
                    drain(2, kT[:, m * c.KL + q4 * 512: m * c.KL + (q4 + 1) * 512],
                          ps[:])
            # Q^T with biases (queries = keys [M, KL) = quarters 2,3)
            for m in range(c.HPT):
                for qc in range(2):
                    ps = psA.tile([128, 512], F32, tag="a", name="psa")
                    for k in range(c.DPT):
                        nc.tensor.matmul(
                            ps[:], wqkv[:, k * 1536 + m * 128: k * 1536 + (m + 1) * 128],
                            xTq[2 + qc][:, ts(k, 512)],
                            start=(k == 0), stop=(k == c.DPT - 1))
                    sl = slice(m * c.Q + qc * 512, m * c.Q + (qc + 1) * 512)
                    nc.scalar.activation(out=rwq[:, sl], in_=ps[:],
                                         func=AF.Identity, bias=rwb_s[:, m:m + 1])
                    nc.vector.tensor_scalar_add(out=rrq[:, sl], in0=ps[:],
                                                scalar1=rrb_s[:, m:m + 1])
            # V natural (+ ones col per head)
            for jt in range(c.NTT):
                ps = psC.tile([128, 512], F32, tag="c", name="psc")
                for k in range(c.DPT):
                    nc.tensor.matmul(
                        ps[:], xTq[jt // 4][:, k * 512 + (jt % 4) * 128:
                                            k * 512 + (jt % 4 + 1) * 128],
                        wqkv[:, k * 1536 + 2 * c.HD: k * 1536 + 3 * c.HD],
                        start=(k == 0), stop=(k == c.DPT - 1))
                dst = bass.AP(
                    tensor=vb.tensor, offset=vb.offset + jt * c.VW,
                    ap=[[c.NTT * c.VW, 128], [65, c.NHC], [1, c.DH]])
                nc.vector.tensor_copy(out=dst, in_=ps[:])
                ones = bass.AP(
                    tensor=vb.tensor, offset=vb.offset + jt * c.VW + c.DH,
                    ap=[[c.NTT * c.VW, 128], [65, c.NHC], [1, 1]])
                nc.vector.memset(ones, 1.0)

        # ============ phase C: attention ============
        w1h = wff1.tile([128, c.DPT * 2048], BF16, tag="w1h")

        def drain(eng_i, out_ap, in_ap):
            if eng_i == 0:
                nc.vector.tensor_copy(out=out_ap, in_=in_ap)
            elif eng_i == 1:
                nc.scalar.activation(out=out_ap, in_=in_ap, func=AF.Copy)
            else:
                n = in_ap.shape[-1]
                h = (n * 5 // 8) & ~63 or n // 2
                nc.vector.tensor_copy(out=out_ap[:, 0:h], in_=in_ap[:, 0:h])
                nc.scalar.activation(out=out_ap[:, h:n], in_=in_ap[:, h:n],
                                     func=AF.Copy)

        BD_ENG = [0, 1, 0, 1]   # DVE, ACT, ...
        AC_ENG = [1, 0, 1, 0]

        with tc.tile_pool(name="score", bufs=4) as score, \
             tc.tile_pool(name="slabp", bufs=5) as slabp:
            owb = wo.tile([128, c.HPT * c.D], BF16, tag="owb")
            nc.gpsimd.dma_start(out=owb[:], in_=bass.AP(
                tensor=io["oww"].ap().tensor, offset=0,
                ap=[[c.D, 128], [128 * c.D, c.HPT], [1, c.D]]))

            pvs = [None, None]
            exp_q = []
            pv_q = []

            def emit_exp(st):
                sb, bdw, jmax = st["sb"], st["bdw"], st["jmax"]
                diag = bass.AP(tensor=bdw.tensor, offset=bdw.offset + 127,
                               ap=[[c.WB - 1, 128], [1, jmax]])
                nc.gpsimd.dma_start(out=sb[:, 0:jmax], in_=diag,
                                    accum_op=ALU.add)
                slab = slabp.tile([128, c.KL], BF16, tag="slab", name="slab")
                for hlo, hhi in [(0, jmax // 256 * 128), (jmax // 256 * 128, jmax)]:
                    nc.scalar.activation(out=sb[:, hlo:hhi], in_=sb[:, hlo:hhi],
                                         func=AF.Exp, scale=float(c.SCALE))
                    dstap = bass.AP(
                        tensor=slab.tensor, offset=slab.offset + hlo,
                        ap=[[c.KL, 128], [128, (hhi - hlo) // 128], [1, 128]])
                    nc.sync.dma_start(out=dstap, in_=sb[:, hlo:hhi],
                                      transpose=True)
                st["slab"] = slab

            def emit_pv(st):
                h, s, jmax, slab = st["h"], st["s"], st["jmax"], st["slab"]
                hp, hr = h // 2, (h % 2) * 64
                njt = jmax // 128
                if s % 4 == 0:
                    pv = pvs[h % 2] = psPV.tile([65, 512], F32, tag="pv",
                                                name="pspv")
                else:
                    pv = pvs[h % 2]
                col = (s % 4) * 128
                for jt in range(njt):
                    nc.tensor.matmul(
                        pv[0:65, col:col + 128],
                        vb[:, jt * c.VW + h * 65: jt * c.VW + h * 65 + 65],
                        slab[:, ts(jt, 128)],
                        start=(jt == 0), stop=(jt == njt - 1))
                if s % 4 == 3:
                    g = s // 4
                    rd = small.tile([1, 512], F32, tag="rd")
                    nc.vector.reciprocal(out=rd[0:1, :], in_=pv[64:65, :])
                    rdb = small.tile([64, 512], F32, tag="rdb")
                    src_b = bass.AP(tensor=rd.tensor, offset=rd.offset,
                                    ap=[[512, 1], [0, 64], [1, 512]])
                    nc.sync.dma_start(out=rdb[:], in_=src_b)
                    nc.vector.tensor_tensor(
                        out=attnT[hr:hr + 64, hp * c.Q + g * 512:
                                  hp * c.Q + (g + 1) * 512],
                        in0=pv[0:64, :], in1=rdb[:], op=ALU.mult)

            for hp in range(c.HPT):
                for s in range(8):
                  for h2 in range(2):
                    h = 2 * hp + h2
                    hr = h2 * 64
                    jmax, wst = c.jmax(s), c.wstart(s)
                    # --- BD window ---
                    bdw = score.tile([128, c.WB], BF16, tag="bdw")
                    for ci, (lo, hi) in enumerate(chunks(jmax)):
                        ps = psB.tile([128, 512], F32, tag="b", name="psbd")
                        nc.tensor.matmul(
                            ps[:, 0:hi - lo],
                            rrq[hr:hr + 64, hp * c.Q + s * 128: hp * c.Q + (s + 1) * 128],
                            rTp[hr:hr + 64, hp * c.KL + wst + lo: hp * c.KL + wst + hi],
                            start=True, stop=True)
                        drain(BD_ENG[h % 2], bdw[:, lo:hi], ps[:, 0:hi - lo])
                    nc.gpsimd.memset(bdw[:, jmax:jmax + 128], c.MASKV)
                    # --- AC ---
                    sb = score.tile([128, c.KL], BF16, tag="sb")
                    for lo, hi in chunks(jmax):
                        ps = psA.tile([128, 512], F32, tag="a", name="psac")
                        nc.tensor.matmul(
                            ps[:, 0:hi - lo],
                            rwq[hr:hr + 64, hp * c.Q + s * 128: hp * c.Q + (s + 1) * 128],
                            kT[hr:hr + 64, hp * c.KL + lo: hp * c.KL + hi],
                            start=True, stop=True)
                        drain(AC_ENG[h % 2], sb[:, lo:hi], ps[:, 0:hi - lo])
                    st = {"h": h, "s": s, "jmax": jmax, "sb": sb, "bdw": bdw}
                    exp_q.append(st)
                    pv_q.append(st)
                    if len(exp_q) > 1:
                        emit_exp(exp_q.pop(0))
                    if len(pv_q) > 2:
                        emit_pv(pv_q.pop(0))
            while exp_q:
                emit_exp(exp_q.pop(0))
            while pv_q:
                emit_pv(pv_q.pop(0))

        attk.release()

        # ============ phase D: o_proj (natural) -> ReduceScatter ============
        with tc.tile_pool(name="stD", bufs=3) as stD:
            for t in range(8):
                ob = stD.tile([128, c.D], BF16, tag="ob")
                for ci in range(2):
                    ps = psA.tile([128, 512], F32, tag="a", name="pso")
                    for k in range(c.HPT):
                        nc.tensor.matmul(
                            ps[:], attnT[:, k * c.Q + t * 128: k * c.Q + (t + 1) * 128],
                            owb[:, k * c.D + ci * 512: k * c.D + (ci + 1) * 512],
                            start=(k == 0), stop=(k == c.HPT - 1))
                    nc.scalar.activation(out=ob[:, ts(ci, 512)], in_=ps[:],
                                         func=AF.Copy)
                nc.sync.dma_start(out=io["rs_bin"][ts(t, 128), :], in_=ob[:])
            if collective:
                nc.gpsimd.collective_compute(
                    "ReduceScatter", ALU.add, replica_groups=rg,
                    ins=[io["rs_bin"].ap().opt()], outs=[io["rs_bout"].ap().opt()])
            else:
                nc.sync.dma_start(out=io["rs_bout"].ap().opt(),
                                  in_=io["rs_bin"].ap()[0:c.TOKF, :].opt())

            # w2 cast-load (during collective window)
            w2t = wff2.tile([128, 32 * c.D], BF16, tag="w2t")
            for g in range(4):
                nc.gpsimd.dma_start(
                    out=w2t[:, g * 8 * c.D:(g + 1) * 8 * c.D],
                    in_=bass.AP(tensor=io["ffw2"].ap().tensor,
                                offset=g * 8 * 128 * c.D,
                                ap=[[c.D, 128], [128 * c.D, 8], [1, c.D]]))
        atp.release()

        # ============ phase E: LN1 + FFN + LN2 ============
        phE = ctx.enter_context(tc.tile_pool(name="phE", bufs=1))
        eps_t = phE.tile([128, 1], F32, tag="eps")
        nc.vector.memset(eps_t[:], c.LN_EPS)
        lns = {}
        for nm in ("ln1g", "ln1b", "ln2g", "ln2b", "ffb2"):
            tl = phE.tile([128, c.D], BF16, tag=nm)
            bcast = bass.AP(tensor=io[nm].ap().tensor, offset=0,
                            ap=[[0, 128], [1, c.D]])
            nc.gpsimd.dma_start(out=tl[:], in_=bcast)
            lns[nm] = tl
        fb1 = phE.tile([128, c.DI // 128], F32, tag="fb1")
        nc.sync.dma_start(out=fb1[:], in_=bass.AP(
            tensor=io["ffb1"].ap().tensor, offset=0,
            ap=[[1, 128], [128, c.DI // 128]]))

        ffn = ctx.enter_context(tc.tile_pool(name="ffn", bufs=1))
        ntt = c.TOKF // 128
        ln1n = ffn.tile([128, ntt * c.D], F32, tag="ln1n")
        lnT = ffn.tile([128, c.DPT * c.TOKF], BF16, tag="lnT")
        hT = ffn.tile([128, 32 * c.TOKF], BF16, tag="hT")

        with tc.tile_pool(name="stE1", bufs=2) as stE1:
            for t in range(ntt):
                zt = stE1.tile([128, c.D], BF16, tag="zt")
                nc.sync.dma_start(out=zt[:], in_=io["rs_bout"][ts(t, 128), :])
                wv = stE1.tile([128, c.D], F32, tag="wv")
                nc.sync.dma_start(out=wv[:], in_=io["wres"][ts(t, 128), :])
                zf = stE1.tile([128, c.D], F32, tag="zf")
                nc.vector.tensor_tensor(out=zf[:], in0=wv[:], in1=zt[:],
                                        op=ALU.add)
                _layernorm_nat(nc, c, small, zf[:], eps_t,
                               lns["ln1g"], lns["ln1b"], ln1n[:, ts(t, c.D)])
                for g in range(2):
                    pst = psB.tile([128, 512], F32, tag="b", name="pstr")
                    for j in range(4):
                        nc.tensor.transpose(
                            pst[:, ts(j, 128)],
                            ln1n[:, t * c.D + (g * 4 + j) * 128:
                                 t * c.D + (g * 4 + j + 1) * 128],
                            identf[:])
                    dst = bass.AP(
                        tensor=lnT.tensor,
                        offset=lnT.offset + g * 4 * c.TOKF + t * 128,
                        ap=[[c.DPT * c.TOKF, 128], [c.TOKF, 4], [1, 128]])
                    nc.vector.tensor_copy(out=dst, in_=pst[:])

        def load_w2g(g):
            w2g = wff2.tile([128, 8 * c.D], BF16, tag="w2g", name="w2g")
            nc.gpsimd.dma_start(
                out=w2g[:],
                in_=bass.AP(tensor=io["ffw2"].ap().tensor,
                            offset=g * 8 * 128 * c.D,
                            ap=[[c.D, 128], [128 * c.D, 8], [1, c.D]]))
            return w2g

        w2cache = {}
        # FFN1: hT[di, tok] = relu(w1^T @ ln1^T + b1); w1 streamed in quarters
        for quarter in range(4):
            w1q = w1q_cur
            if quarter < 3:
                w1q_cur = load_w1q(quarter + 1)
            if quarter >= 2:
                g = quarter - 2
                w2cache[g] = load_w2g(g)
            for mm in range(8):
                m = quarter * 8 + mm
                pp, tg = (psA, "a") if mm % 2 == 0 else (psB, "b")
                ps = pp.tile([128, 512], F32, tag=tg, name="psf1")
                for k in range(c.DPT):
                    nc.tensor.matmul(
                        ps[:], w1q[:, k * 1024 + mm * 128: k * 1024 + (mm + 1) * 128],
                        lnT[:, ts(k, c.TOKF)],
                        start=(k == 0), stop=(k == c.DPT - 1))
                nc.scalar.activation(
                    out=hT[:, ts(m, c.TOKF)], in_=ps[:],
                    func=AF.Relu, bias=fb1[:, m:m + 1])

        # FFN2: natural out = hT^T @ w2 (+ residual + b2), LN2
        with tc.tile_pool(name="stE2", bufs=2) as stE2:
            for t in range(ntt):
                o2n = stE2.tile([128, c.D], F32, tag="o2n")
                for ci in range(2):
                    pp, tg = (psA, "a") if ci == 0 else (psC, "c")
                    ps = pp.tile([128, 512], F32, tag=tg, name="psf2")
                    for m in range(32):
                        nc.tensor.matmul(
                            ps[:], hT[:, m * c.TOKF + t * 128: m * c.TOKF + (t + 1) * 128],
                            w2t[:, m * c.D + ci * 512: m * c.D + (ci + 1) * 512],
                            start=(m == 0), stop=(m == 31))
                    nc.vector.tensor_tensor(
                        out=o2n[:, ts(ci, 512)], in0=ps[:],
                        in1=ln1n[:, t * c.D + ci * 512: t * c.D + (ci + 1) * 512],
                        op=ALU.add)
                nc.vector.tensor_tensor(out=o2n[:], in0=o2n[:],
                                        in1=lns["ffb2"][:], op=ALU.add)
                fin = stE2.tile([128, c.D], F32, tag="fin")
                _layernorm_nat(nc, c, small, o2n[:], eps_t,
                               lns["ln2g"], lns["ln2b"], fin[:])
                nc.sync.dma_start(out=io["out"][ts(t, 128), :], in_=fin[:])


def _layernorm_nat(nc, c, small, z, eps_t, g, b, out_dst):
    """LayerNorm over the free axis of z [128, D] fp32."""
    BN_FMAX = nc.vector.BN_STATS_FMAX
    d = z.shape[-1]
    sub = math.gcd(BN_FMAX, d)
    nsub = d // sub
    zr = z.rearrange("p (n f) -> p n f", f=sub)
    stats = small.tile([128, nsub, nc.vector.BN_STATS_DIM], F32, tag="bnst")
    for i in range(nsub):
        nc.vector.bn_stats(out=stats[:, i, :], in_=zr[:, i, :])
    mv = small.tile([128, nc.vector.BN_AGGR_DIM], F32, tag="bnag")
    nc.vector.bn_aggr(out=mv[:], in_=stats[:])
    mean, var = mv[:, 0:1], mv[:, 1:2]
    nc.scalar.activation(out=var, in_=var, func=AF.Sqrt, bias=eps_t[:], scale=1.0)
    nc.vector.reciprocal(out=var, in_=var)
    nc.vector.tensor_scalar(out=out_dst, in0=z, scalar1=mean, scalar2=var,
                            op0=ALU.subtract, op1=ALU.mult)
    nc.vector.tensor_tensor(out=out_dst, in0=out_dst, in1=g[:, 0:d], op=ALU.mult)
    nc.vector.tensor_tensor(out=out_dst, in0=out_dst, in1=b[:, 0:d], op=ALU.add)


# ============================================================
# host-side sharding + entry point
# ============================================================

def shard_inputs(inputs, c: Cfg = None):
    c = c or Cfg()
    w = np.asarray(inputs["w"], np.float32)
    r = np.asarray(inputs["r"], np.float32)
    mems = np.asarray(inputs["mems"], np.float32)
    qkv_w = np.asarray(inputs["qkv_w"], np.float32)
    r_net_w = np.asarray(inputs["r_net_w"], np.float32)
    o_w = np.asarray(inputs["o_w"], np.float32)
    r_w_bias = np.asarray(inputs["r_w_bias"], np.float32).reshape(-1)
    r_r_bias = np.asarray(inputs["r_r_bias"], np.float32).reshape(-1)
    NHD = qkv_w.shape[1] // 3
    in_maps = []
    for core in range(c.N_CORES):
        b, hh = core // 2, core % 2
        hsl = slice(hh * c.HD, (hh + 1) * c.HD)
        xw_c = np.concatenate([mems[:, b, :], w[:, b, :]], axis=0)
        qkvw_c = np.concatenate([qkv_w[:, j * NHD + hh * c.HD:
                                       j * NHD + (hh + 1) * c.HD]
                                 for j in range(3)], axis=1)
        in_maps.append({
            "xw": np.ascontiguousarray(xw_c),
            "r_in": np.ascontiguousarray(r[:, 0, :]),
            "qkvw": np.ascontiguousarray(qkvw_c),
            "rnetw": np.ascontiguousarray(r_net_w[:, hsl]),
            "oww": np.ascontiguousarray(o_w[hsl, :]),
            "rwb": np.ascontiguousarray(r_w_bias[hsl][None, :]),
            "rrb": np.ascontiguousarray(r_r_bias[hsl][None, :]),
            "ln1g": np.asarray(inputs["ln1_g"], np.float32).reshape(1, -1),
            "ln1b": np.asarray(inputs["ln1_b"], np.float32).reshape(1, -1),
            "ln2g": np.asarray(inputs["ln2_g"], np.float32).reshape(1, -1),
            "ln2b": np.asarray(inputs["ln2_b"], np.float32).reshape(1, -1),
            "ffw1": np.asarray(inputs["ff_w1"], np.float32),
            "ffb1": np.asarray(inputs["ff_b1"], np.float32).reshape(1, -1),
            "ffw2": np.asarray(inputs["ff_w2"], np.float32),
            "ffb2": np.asarray(inputs["ff_b2"], np.float32).reshape(1, -1),
            "wres": np.ascontiguousarray(w[hh * c.TOKF:(hh + 1) * c.TOKF, b, :]),
        })
    return in_maps


def unshard_output(results, inputs, c: Cfg = None):
    c = c or Cfg()
    w = np.asarray(inputs["w"])
    Q, B, D = w.shape
    out = np.zeros((Q, B, D), np.float32)
    for core in range(c.N_CORES):
        b, hh = core // 2, core % 2
        out[hh * c.TOKF:(hh + 1) * c.TOKF, b, :] = results[core]["out"]
    return out


_NC_CACHE = {}


def kernel(**inputs):
    if "nc" not in _NC_CACHE:
        _NC_CACHE["nc"] = build_kernel()
    nc = _NC_CACHE["nc"]
    in_maps = shard_inputs(inputs)
    from concourse.bass_utils import run_bass_kernel_spmd
    res = run_bass_kernel_spmd(nc, in_maps, core_ids=list(range(Cfg.N_CORES)))
    return unshard_output(res.results, inputs)


# revision 62
# speedup vs baseline: 1.0095x; 1.0095x over previous
"""Trainium2 Bass kernel for nn_MemTransformerLM (Transformer-XL layer).

Sharding (8 cores): batch (4) x head-half (2). Core c handles batch b = c//2
and heads [hh*8, hh*8+8), hh = c%2, for all 1024 queries. After o_proj a
2-rank bf16 ReduceScatter over core pairs (2b, 2b+1) splits tokens for the
FFN: even core keeps tokens [0,512), odd [512,1024).

Key structure:
- All big f32 DRAM inputs are loaded with SWDGE cast-DMA (f32 -> bf16).
- r^T / x^T built via SBUF->SBUF DMA transpose in key-quarters so projection
  matmuls start while later quarters still stream.
- Scores per (head h, q-tile s): BD window matmuls -> PSUM -> drain to bdw
  (window buffer, masked tail memset to -240); AC matmuls -> PSUM -> drain to
  sb; a "diagonal" SBUF->SBUF accum-DMA adds the rel-shifted BD window onto
  sb; one ACT pass computes exp(scale * sb) in place; DMA-transpose into a
  per-s slab; PV accumulates slab tiles over the unmasked j range only.
- o_proj and FFN run in natural token-major orientation (attnT / hT used as
  lhsT), so no fp32 PE transposes and no strided weight loads.
"""

import contextlib
import math

import numpy as np

import concourse.bass as bass
import concourse.bacc as bacc
import concourse.mybir as mybir
import concourse.tile as tile
from concourse.masks import make_identity

F32 = mybir.dt.float32
BF16 = mybir.dt.bfloat16
AF = mybir.ActivationFunctionType
ALU = mybir.AluOpType


class Cfg:
    D = 1024      # model dim
    NHC = 8       # heads per core
    DH = 64       # head dim
    KL = 2048     # key length
    Q = 1024      # query length
    DI = 4096     # ffn inner
    LN_EPS = 1e-5
    N_CORES = 8

    HD = 512          # head dims per core (NHC * DH)
    M = 1024          # mem length
    TOKF = 512        # ffn tokens per core
    WB = 2176         # bdw window buffer width (KL + 128)
    SCALE = 0.125     # 1/sqrt(DH)
    MASKV = -240.0    # pre-scale mask value: exp(SCALE * -240) ~ 1e-13
    DPT = 8           # D / 128
    HPT = 4           # HD / 128
    NTT = 16          # KL / 128
    VW = 520          # NHC * 65

    def jmax(self, s):
        return 1152 + 128 * s

    def wstart(self, s):
        return 896 - 128 * s


def ts(i, n):
    return slice(i * n, (i + 1) * n)


def chunks(total, sz=512):
    return [(lo, min(total, lo + sz)) for lo in range(0, total, sz)]


def build_kernel(c: Cfg = None, collective=True):
    c = c or Cfg()
    nc = bacc.Bacc("TRN2", target_bir_lowering=False)

    io = {}
    def din(name, shape, dt=F32):
        io[name] = nc.dram_tensor(name, shape, dt, kind="ExternalInput")
    din("xw", [c.KL, c.D])
    din("r_in", [c.KL, c.D])
    din("qkvw", [c.D, 3 * c.HD])
    din("rnetw", [c.D, c.HD])
    din("oww", [c.HD, c.D])
    din("rwb", [1, c.HD])
    din("rrb", [1, c.HD])
    din("ln1g", [1, c.D]); din("ln1b", [1, c.D])
    din("ln2g", [1, c.D]); din("ln2b", [1, c.D])
    din("ffw1", [c.D, c.DI]); din("ffb1", [1, c.DI])
    din("ffw2", [c.DI, c.D]); din("ffb2", [1, c.D])
    din("wres", [c.TOKF, c.D])
    io["out"] = nc.dram_tensor("out", [c.TOKF, c.D], F32, kind="ExternalOutput")
    io["rs_bin"] = nc.dram_tensor("rs_bin", [c.Q, c.D], BF16)
    io["rs_bout"] = nc.dram_tensor("rs_bout", [c.TOKF, c.D], BF16)

    with tile.TileContext(nc) as tc:
        _body(tc, nc, c, io, collective=collective)
    nc.finalize()
    return nc


def _body(tc, nc, c, io, collective=True):
    ctx = contextlib.ExitStack()
    rg = [[i, i + 1] for i in range(0, c.N_CORES, 2)]
    with ctx:
        small = ctx.enter_context(tc.tile_pool(name="small", bufs=4))
        psA = ctx.enter_context(tc.tile_pool(name="psA", bufs=3, space="PSUM"))
        psB = ctx.enter_context(tc.tile_pool(name="psB", bufs=3, space="PSUM"))
        psPV = ctx.enter_context(tc.tile_pool(name="psPV", bufs=2, space="PSUM"))

        keep = ctx.enter_context(tc.tile_pool(name="keep", bufs=1))
        ident = keep.tile([128, 128], BF16, tag="identb")
        make_identity(nc, ident)
        identf = keep.tile([128, 128], F32, tag="identf")
        make_identity(nc, identf)

        # biases: rwb/rrb as [128, HPT] (partition = dh within head pair col)
        rwb_s = keep.tile([128, c.HPT], F32, tag="rwb")
        rrb_s = keep.tile([128, c.HPT], F32, tag="rrb")

        # pools created in stack order (release: attk -> atp; rest at exit)
        wo = ctx.enter_context(tc.tile_pool(name="wo", bufs=1))
        wff1 = ctx.enter_context(tc.tile_pool(name="wff1", bufs=1))
        wff2 = ctx.enter_context(tc.tile_pool(name="wff2", bufs=1))
        atp = tc.alloc_tile_pool(name="atp", bufs=1)
        attnT = atp.tile([128, c.HPT * c.Q], BF16, tag="attnT")
        # long-lived attention operands
        attk = tc.alloc_tile_pool(name="attk", bufs=1)
        rTp = attk.tile([128, c.HPT * c.KL], BF16, tag="rTp")
        kT = attk.tile([128, c.HPT * c.KL], BF16, tag="kT")
        vb = attk.tile([128, c.NTT * c.VW], BF16, tag="vb")
        rwq = attk.tile([128, c.HPT * c.Q], BF16, tag="rwq")
        rrq = attk.tile([128, c.HPT * c.Q], BF16, tag="rrq")

        # ============ phase AB: loads + projections, pipelined ============
        with tc.tile_pool(name="phAB", bufs=1) as phAB, \
             tc.tile_pool(name="stAB", bufs=3) as stAB:
            rTq = [phAB.tile([128, c.DPT * 512], BF16, tag="rTq%d" % q,
                             name="rTq%d" % q) for q in range(4)]
            xTq = [phAB.tile([128, c.DPT * 512], BF16, tag="xTq%d" % q,
                             name="xTq%d" % q) for q in range(4)]
            wr = phAB.tile([128, c.DPT * c.HD], BF16, tag="wr")
            wqkv = phAB.tile([128, c.DPT * 3 * c.HD], BF16, tag="wqkv")

            nc.gpsimd.dma_start(out=wr[:], in_=bass.AP(
                tensor=io["rnetw"].ap().tensor, offset=0,
                ap=[[c.HD, 128], [128 * c.HD, c.DPT], [1, c.HD]]))
            nc.sync.dma_start(out=rwb_s[:], in_=bass.AP(
                tensor=io["rwb"].ap().tensor, offset=0, ap=[[1, 128], [128, c.HPT]]))
            nc.sync.dma_start(out=rrb_s[:], in_=bass.AP(
                tensor=io["rrb"].ap().tensor, offset=0, ap=[[1, 128], [128, c.HPT]]))
            nc.gpsimd.dma_start(out=wqkv[:], in_=bass.AP(
                tensor=io["qkvw"].ap().tensor, offset=0,
                ap=[[3 * c.HD, 128], [128 * 3 * c.HD, c.DPT], [1, 3 * c.HD]]))

            def load_transposed(src, dstq):
                # 8 cast-DMAs of 2 key-tiles each; transpose per key-tile
                for g in range(8):
                    nat = stAB.tile([128, 2 * c.D], BF16, tag="nat")
                    nc.gpsimd.dma_start(out=nat[:], in_=bass.AP(
                        tensor=src.ap().tensor, offset=g * 2 * 128 * c.D,
                        ap=[[c.D, 128], [128 * c.D, 2], [1, c.D]]))
                    for h2 in range(2):
                        tt = g * 2 + h2
                        dst = dstq[tt // 4]
                        dstap = bass.AP(
                            tensor=dst.tensor,
                            offset=dst.offset + (tt % 4) * 128,
                            ap=[[c.DPT * 512, 128], [512, c.DPT], [1, 128]])
                        nc.sync.dma_start(
                            out=dstap, in_=nat[:, ts(h2, c.D)], transpose=True)

            load_transposed(io["r_in"], rTq)
            load_transposed(io["xw"], xTq)

            # rTp: r_head_k^T per head-pair m
            for m in range(c.HPT):
                for q4 in range(4):
                    ps = psA.tile([128, 512], F32, tag="a", name="psa")
                    for k in range(c.DPT):
                        nc.tensor.matmul(
                            ps[:], wr[:, k * c.HD + m * 128: k * c.HD + (m + 1) * 128],
                            rTq[q4][:, ts(k, 512)],
                            start=(k == 0), stop=(k == c.DPT - 1))
                    nc.vector.tensor_copy(
                        out=rTp[:, m * c.KL + q4 * 512: m * c.KL + (q4 + 1) * 512],
                        in_=ps[:])
            # K^T
            for m in range(c.HPT):
                for q4 in range(4):
                    ps = psB.tile([128, 512], F32, tag="b", name="psb")
                    for k in range(c.DPT):
                        nc.tensor.matmul(
                            ps[:], wqkv[:, k * 1536 + c.HD + m * 128:
                                        k * 1536 + c.HD + (m + 1) * 128],
                            xTq[q4][:, ts(k, 512)],
                            start=(k == 0), stop=(k == c.DPT - 1))
                    drain(2, kT[:, m * c.KL + q4 * 512: m * c.KL + (q4 + 1) * 512],
                          ps[:])
            # Q^T with biases (queries = keys [M, KL) = quarters 2,3)
            for m in range(c.HPT):
                for qc in range(2):
                    ps = psA.tile([128, 512], F32, tag="a", name="psa")
                    for k in range(c.DPT):
                        nc.tensor.matmul(
                            ps[:], wqkv[:, k * 1536 + m * 128: k * 1536 + (m + 1) * 128],
                            xTq[2 + qc][:, ts(k, 512)],
                            start=(k == 0), stop=(k == c.DPT - 1))
                    sl = slice(m * c.Q + qc * 512, m * c.Q + (qc + 1) * 512)
                    nc.scalar.activation(out=rwq[:, sl], in_=ps[:],
                                         func=AF.Identity, bias=rwb_s[:, m:m + 1])
                    nc.vector.tensor_scalar_add(out=rrq[:, sl], in0=ps[:],
                                                scalar1=rrb_s[:, m:m + 1])
            # V natural (+ ones col per head)
            for jt in range(c.NTT):
                ps = psC.tile([128, 512], F32, tag="c", name="psc")
                for k in range(c.DPT):
                    nc.tensor.matmul(
                        ps[:], xTq[jt // 4][:, k * 512 + (jt % 4) * 128:
                                            k * 512 + (jt % 4 + 1) * 128],
                        wqkv[:, k * 1536 + 2 * c.HD: k * 1536 + 3 * c.HD],
                        start=(k == 0), stop=(k == c.DPT - 1))
                dst = bass.AP(
                    tensor=vb.tensor, offset=vb.offset + jt * c.VW,
                    ap=[[c.NTT * c.VW, 128], [65, c.NHC], [1, c.DH]])
                nc.vector.tensor_copy(out=dst, in_=ps[:])
                ones = bass.AP(
                    tensor=vb.tensor, offset=vb.offset + jt * c.VW + c.DH,
                    ap=[[c.NTT * c.VW, 128], [65, c.NHC], [1, 1]])
                nc.vector.memset(ones, 1.0)

        # ============ phase C: attention ============
        w1h = wff1.tile([128, c.DPT * 2048], BF16, tag="w1h")

        def drain(eng_i, out_ap, in_ap):
            if eng_i == 0:
                nc.vector.tensor_copy(out=out_ap, in_=in_ap)
            elif eng_i == 1:
                nc.scalar.activation(out=out_ap, in_=in_ap, func=AF.Copy)
            else:
                n = in_ap.shape[-1]
                h = (n * 5 // 8) & ~63 or n // 2
                nc.vector.tensor_copy(out=out_ap[:, 0:h], in_=in_ap[:, 0:h])
                nc.scalar.activation(out=out_ap[:, h:n], in_=in_ap[:, h:n],
                                     func=AF.Copy)

        BD_ENG = [0, 1, 0, 1]   # DVE, ACT, ...
        AC_ENG = [1, 0, 1, 0]

        with tc.tile_pool(name="score", bufs=4) as score, \
             tc.tile_pool(name="slabp", bufs=5) as slabp:
            owb = wo.tile([128, c.HPT * c.D], BF16, tag="owb")
            nc.gpsimd.dma_start(out=owb[:], in_=bass.AP(
                tensor=io["oww"].ap().tensor, offset=0,
                ap=[[c.D, 128], [128 * c.D, c.HPT], [1, c.D]]))

            pvs = [None, None]
            exp_q = []
            pv_q = []

            def emit_exp(st):
                sb, bdw, jmax = st["sb"], st["bdw"], st["jmax"]
                diag = bass.AP(tensor=bdw.tensor, offset=bdw.offset + 127,
                               ap=[[c.WB - 1, 128], [1, jmax]])
                nc.gpsimd.dma_start(out=sb[:, 0:jmax], in_=diag,
                                    accum_op=ALU.add)
                slab = slabp.tile([128, c.KL], BF16, tag="slab", name="slab")
                for hlo, hhi in [(0, jmax // 256 * 128), (jmax // 256 * 128, jmax)]:
                    nc.scalar.activation(out=sb[:, hlo:hhi], in_=sb[:, hlo:hhi],
                                         func=AF.Exp, scale=float(c.SCALE))
                    dstap = bass.AP(
                        tensor=slab.tensor, offset=slab.offset + hlo,
                        ap=[[c.KL, 128], [128, (hhi - hlo) // 128], [1, 128]])
                    nc.sync.dma_start(out=dstap, in_=sb[:, hlo:hhi],
                                      transpose=True)
                st["slab"] = slab

            def emit_pv(st):
                h, s, jmax, slab = st["h"], st["s"], st["jmax"], st["slab"]
                hp, hr = h // 2, (h % 2) * 64
                njt = jmax // 128
                if s % 4 == 0:
                    pv = pvs[h % 2] = psPV.tile([65, 512], F32, tag="pv",
                                                name="pspv")
                else:
                    pv = pvs[h % 2]
                col = (s % 4) * 128
                for jt in range(njt):
                    nc.tensor.matmul(
                        pv[0:65, col:col + 128],
                        vb[:, jt * c.VW + h * 65: jt * c.VW + h * 65 + 65],
                        slab[:, ts(jt, 128)],
                        start=(jt == 0), stop=(jt == njt - 1))
                if s % 4 == 3:
                    g = s // 4
                    rd = small.tile([1, 512], F32, tag="rd")
                    nc.vector.reciprocal(out=rd[0:1, :], in_=pv[64:65, :])
                    rdb = small.tile([64, 512], F32, tag="rdb")
                    src_b = bass.AP(tensor=rd.tensor, offset=rd.offset,
                                    ap=[[512, 1], [0, 64], [1, 512]])
                    nc.sync.dma_start(out=rdb[:], in_=src_b)
                    nc.vector.tensor_tensor(
                        out=attnT[hr:hr + 64, hp * c.Q + g * 512:
                                  hp * c.Q + (g + 1) * 512],
                        in0=pv[0:64, :], in1=rdb[:], op=ALU.mult)

            for hp in range(c.HPT):
                for s in range(8):
                  for h2 in range(2):
                    h = 2 * hp + h2
                    hr = h2 * 64
                    jmax, wst = c.jmax(s), c.wstart(s)
                    # --- BD window ---
                    bdw = score.tile([128, c.WB], BF16, tag="bdw")
                    for ci, (lo, hi) in enumerate(chunks(jmax)):
                        ps = psB.tile([128, 512], F32, tag="b", name="psbd")
                        nc.tensor.matmul(
                            ps[:, 0:hi - lo],
                            rrq[hr:hr + 64, hp * c.Q + s * 128: hp * c.Q + (s + 1) * 128],
                            rTp[hr:hr + 64, hp * c.KL + wst + lo: hp * c.KL + wst + hi],
                            start=True, stop=True)
                        drain(BD_ENG[h % 2], bdw[:, lo:hi], ps[:, 0:hi - lo])
                    nc.gpsimd.memset(bdw[:, jmax:jmax + 128], c.MASKV)
                    # --- AC ---
                    sb = score.tile([128, c.KL], BF16, tag="sb")
                    for lo, hi in chunks(jmax):
                        ps = psA.tile([128, 512], F32, tag="a", name="psac")
                        nc.tensor.matmul(
                            ps[:, 0:hi - lo],
                            rwq[hr:hr + 64, hp * c.Q + s * 128: hp * c.Q + (s + 1) * 128],
                            kT[hr:hr + 64, hp * c.KL + lo: hp * c.KL + hi],
                            start=True, stop=True)
                        drain(AC_ENG[h % 2], sb[:, lo:hi], ps[:, 0:hi - lo])
                    st = {"h": h, "s": s, "jmax": jmax, "sb": sb, "bdw": bdw}
                    exp_q.append(st)
                    pv_q.append(st)
                    if len(exp_q) > 1:
                        emit_exp(exp_q.pop(0))
                    if len(pv_q) > 2:
                        emit_pv(pv_q.pop(0))
            while exp_q:
                emit_exp(exp_q.pop(0))
            while pv_q:
                emit_pv(pv_q.pop(0))

        attk.release()

        # ============ phase D: o_proj (natural) -> ReduceScatter ============
        with tc.tile_pool(name="stD", bufs=4) as stD:
            for t in range(8):
                ob = stD.tile([128, c.D], BF16, tag="ob")
                for ci in range(2):
                    pp, tg = (psA, "a") if ci == 0 else (psB, "b")
                    ps = pp.tile([128, 512], F32, tag=tg, name="pso")
                    for k in range(c.HPT):
                        nc.tensor.matmul(
                            ps[:], attnT[:, k * c.Q + t * 128: k * c.Q + (t + 1) * 128],
                            owb[:, k * c.D + ci * 512: k * c.D + (ci + 1) * 512],
                            start=(k == 0), stop=(k == c.HPT - 1))
                    nc.scalar.activation(out=ob[:, ts(ci, 512)], in_=ps[:],
                                         func=AF.Copy)
                nc.sync.dma_start(out=io["rs_bin"][ts(t, 128), :], in_=ob[:])
            if collective:
                nc.gpsimd.collective_compute(
                    "ReduceScatter", ALU.add, replica_groups=rg,
                    ins=[io["rs_bin"].ap().opt()], outs=[io["rs_bout"].ap().opt()])
            else:
                nc.sync.dma_start(out=io["rs_bout"].ap().opt(),
                                  in_=io["rs_bin"].ap()[0:c.TOKF, :].opt())

            # w2 cast-load (during collective window)
            w2t = wff2.tile([128, 32 * c.D], BF16, tag="w2t")
            for g in range(4):
                nc.gpsimd.dma_start(
                    out=w2t[:, g * 8 * c.D:(g + 1) * 8 * c.D],
                    in_=bass.AP(tensor=io["ffw2"].ap().tensor,
                                offset=g * 8 * 128 * c.D,
                                ap=[[c.D, 128], [128 * c.D, 8], [1, c.D]]))
        atp.release()

        # ============ phase E: LN1 + FFN + LN2 ============
        phE = ctx.enter_context(tc.tile_pool(name="phE", bufs=1))
        eps_t = phE.tile([128, 1], F32, tag="eps")
        nc.vector.memset(eps_t[:], c.LN_EPS)
        lns = {}
        for nm in ("ln1g", "ln1b", "ln2g", "ln2b", "ffb2"):
            tl = phE.tile([128, c.D], BF16, tag=nm)
            bcast = bass.AP(tensor=io[nm].ap().tensor, offset=0,
                            ap=[[0, 128], [1, c.D]])
            nc.gpsimd.dma_start(out=tl[:], in_=bcast)
            lns[nm] = tl
        fb1 = phE.tile([128, c.DI // 128], F32, tag="fb1")
        nc.sync.dma_start(out=fb1[:], in_=bass.AP(
            tensor=io["ffb1"].ap().tensor, offset=0,
            ap=[[1, 128], [128, c.DI // 128]]))

        ffn = ctx.enter_context(tc.tile_pool(name="ffn", bufs=1))
        ntt = c.TOKF // 128
        ln1n = ffn.tile([128, ntt * c.D], F32, tag="ln1n")
        lnT = ffn.tile([128, c.DPT * c.TOKF], BF16, tag="lnT")
        hT = ffn.tile([128, 32 * c.TOKF], BF16, tag="hT")

        with tc.tile_pool(name="stE1", bufs=4) as stE1:
            for t in range(ntt):
                zt = stE1.tile([128, c.D], BF16, tag="zt")
                nc.sync.dma_start(out=zt[:], in_=io["rs_bout"][ts(t, 128), :])
                wv = stE1.tile([128, c.D], F32, tag="wv")
                nc.sync.dma_start(out=wv[:], in_=io["wres"][ts(t, 128), :])
                zf = stE1.tile([128, c.D], F32, tag="zf")
                nc.vector.tensor_tensor(out=zf[:], in0=wv[:], in1=zt[:],
                                        op=ALU.add)
                _layernorm_nat(nc, c, small, zf[:], eps_t,
                               lns["ln1g"], lns["ln1b"], ln1n[:, ts(t, c.D)])
                for g in range(2):
                    pst = psB.tile([128, 512], F32, tag="b", name="pstr")
                    for j in range(4):
                        nc.tensor.transpose(
                            pst[:, ts(j, 128)],
                            ln1n[:, t * c.D + (g * 4 + j) * 128:
                                 t * c.D + (g * 4 + j + 1) * 128],
                            identf[:])
                    dst = bass.AP(
                        tensor=lnT.tensor,
                        offset=lnT.offset + g * 4 * c.TOKF + t * 128,
                        ap=[[c.DPT * c.TOKF, 128], [c.TOKF, 4], [1, 128]])
                    nc.vector.tensor_copy(out=dst, in_=pst[:])

        def load_w2g(g):
            w2g = wff2.tile([128, 8 * c.D], BF16, tag="w2g", name="w2g")
            nc.gpsimd.dma_start(
                out=w2g[:],
                in_=bass.AP(tensor=io["ffw2"].ap().tensor,
                            offset=g * 8 * 128 * c.D,
                            ap=[[c.D, 128], [128 * c.D, 8], [1, c.D]]))
            return w2g

        w2cache = {}
        # FFN1: hT[di, tok] = relu(w1^T @ ln1^T + b1); w1 streamed in quarters
        for quarter in range(4):
            w1q = w1q_cur
            if quarter < 3:
                w1q_cur = load_w1q(quarter + 1)
            if quarter >= 2:
                g = quarter - 2
                w2cache[g] = load_w2g(g)
            for mm in range(8):
                m = quarter * 8 + mm
                pp, tg = (psA, "a") if mm % 2 == 0 else (psB, "b")
                ps = pp.tile([128, 512], F32, tag=tg, name="psf1")
                for k in range(c.DPT):
                    nc.tensor.matmul(
                        ps[:], w1q[:, k * 1024 + mm * 128: k * 1024 + (mm + 1) * 128],
                        lnT[:, ts(k, c.TOKF)],
                        start=(k == 0), stop=(k == c.DPT - 1))
                nc.scalar.activation(
                    out=hT[:, ts(m, c.TOKF)], in_=ps[:],
                    func=AF.Relu, bias=fb1[:, m:m + 1])

        # FFN2: natural out = hT^T @ w2 (+ residual + b2), LN2
        with tc.tile_pool(name="stE2", bufs=4) as stE2:
            for t in range(ntt):
                o2n = stE2.tile([128, c.D], F32, tag="o2n")
                for ci in range(2):
                    pp, tg = (psA, "a") if ci == 0 else (psC, "c")
                    ps = pp.tile([128, 512], F32, tag=tg, name="psf2")
                    for m in range(32):
                        nc.tensor.matmul(
                            ps[:], hT[:, m * c.TOKF + t * 128: m * c.TOKF + (t + 1) * 128],
                            w2t[:, m * c.D + ci * 512: m * c.D + (ci + 1) * 512],
                            start=(m == 0), stop=(m == 31))
                    nc.vector.tensor_tensor(
                        out=o2n[:, ts(ci, 512)], in0=ps[:],
                        in1=ln1n[:, t * c.D + ci * 512: t * c.D + (ci + 1) * 512],
                        op=ALU.add)
                nc.vector.tensor_tensor(out=o2n[:], in0=o2n[:],
                                        in1=lns["ffb2"][:], op=ALU.add)
                fin = stE2.tile([128, c.D], F32, tag="fin")
                _layernorm_nat(nc, c, small, o2n[:], eps_t,
                               lns["ln2g"], lns["ln2b"], fin[:])
                nc.sync.dma_start(out=io["out"][ts(t, 128), :], in_=fin[:])


def _layernorm_nat(nc, c, small, z, eps_t, g, b, out_dst):
    """LayerNorm over the free axis of z [128, D] fp32."""
    BN_FMAX = nc.vector.BN_STATS_FMAX
    d = z.shape[-1]
    sub = math.gcd(BN_FMAX, d)
    nsub = d // sub
    zr = z.rearrange("p (n f) -> p n f", f=sub)
    stats = small.tile([128, nsub, nc.vector.BN_STATS_DIM], F32, tag="bnst")
    for i in range(nsub):
        nc.vector.bn_stats(out=stats[:, i, :], in_=zr[:, i, :])
    mv = small.tile([128, nc.vector.BN_AGGR_DIM], F32, tag="bnag")
    nc.vector.bn_aggr(out=mv[:], in_=stats[:])
    mean, var = mv[:, 0:1], mv[:, 1:2]
    nc.scalar.activation(out=var, in_=var, func=AF.Sqrt, bias=eps_t[:], scale=1.0)
    nc.vector.reciprocal(out=var, in_=var)
    nc.vector.tensor_scalar(out=out_dst, in0=z, scalar1=mean, scalar2=var,
                            op0=ALU.subtract, op1=ALU.mult)
    nc.vector.tensor_tensor(out=out_dst, in0=out_dst, in1=g[:, 0:d], op=ALU.mult)
    nc.vector.tensor_tensor(out=out_dst, in0=out_dst, in1=b[:, 0:d], op=ALU.add)


# ============================================================
# host-side sharding + entry point
# ============================================================

def shard_inputs(inputs, c: Cfg = None):
    c = c or Cfg()
    w = np.asarray(inputs["w"], np.float32)
    r = np.asarray(inputs["r"], np.float32)
    mems = np.asarray(inputs["mems"], np.float32)
    qkv_w = np.asarray(inputs["qkv_w"], np.float32)
    r_net_w = np.asarray(inputs["r_net_w"], np.float32)
    o_w = np.asarray(inputs["o_w"], np.float32)
    r_w_bias = np.asarray(inputs["r_w_bias"], np.float32).reshape(-1)
    r_r_bias = np.asarray(inputs["r_r_bias"], np.float32).reshape(-1)
    NHD = qkv_w.shape[1] // 3
    in_maps = []
    for core in range(c.N_CORES):
        b, hh = core // 2, core % 2
        hsl = slice(hh * c.HD, (hh + 1) * c.HD)
        xw_c = np.concatenate([mems[:, b, :], w[:, b, :]], axis=0)
        qkvw_c = np.concatenate([qkv_w[:, j * NHD + hh * c.HD:
                                       j * NHD + (hh + 1) * c.HD]
                                 for j in range(3)], axis=1)
        in_maps.append({
            "xw": np.ascontiguousarray(xw_c),
            "r_in": np.ascontiguousarray(r[:, 0, :]),
            "qkvw": np.ascontiguousarray(qkvw_c),
            "rnetw": np.ascontiguousarray(r_net_w[:, hsl]),
            "oww": np.ascontiguousarray(o_w[hsl, :]),
            "rwb": np.ascontiguousarray(r_w_bias[hsl][None, :]),
            "rrb": np.ascontiguousarray(r_r_bias[hsl][None, :]),
            "ln1g": np.asarray(inputs["ln1_g"], np.float32).reshape(1, -1),
            "ln1b": np.asarray(inputs["ln1_b"], np.float32).reshape(1, -1),
            "ln2g": np.asarray(inputs["ln2_g"], np.float32).reshape(1, -1),
            "ln2b": np.asarray(inputs["ln2_b"], np.float32).reshape(1, -1),
            "ffw1": np.asarray(inputs["ff_w1"], np.float32),
            "ffb1": np.asarray(inputs["ff_b1"], np.float32).reshape(1, -1),
            "ffw2": np.asarray(inputs["ff_w2"], np.float32),
            "ffb2": np.asarray(inputs["ff_b2"], np.float32).reshape(1, -1),
            "wres": np.ascontiguousarray(w[hh * c.TOKF:(hh + 1) * c.TOKF, b, :]),
        })
    return in_maps


def unshard_output(results, inputs, c: Cfg = None):
    c = c or Cfg()
    w = np.asarray(inputs["w"])
    Q, B, D = w.shape
    out = np.zeros((Q, B, D), np.float32)
    for core in range(c.N_CORES):
        b, hh = core // 2, core % 2
        out[hh * c.TOKF:(hh + 1) * c.TOKF, b, :] = results[core]["out"]
    return out


_NC_CACHE = {}


def kernel(**inputs):
    if "nc" not in _NC_CACHE:
        _NC_CACHE["nc"] = build_kernel()
    nc = _NC_CACHE["nc"]
    in_maps = shard_inputs(inputs)
    from concourse.bass_utils import run_bass_kernel_spmd
    res = run_bass_kernel_spmd(nc, in_maps, core_ids=list(range(Cfg.N_CORES)))
    return unshard_output(res.results, inputs)
